# revision 1
# baseline (speedup 1.0000x reference)
"""Trilinear 3D grid-encoding lookup on 8 TRN2 NeuronCores.

Strategy (data-parallel, per the sharding hint):
  - Host: build a corner-expanded fp16 table E[v, 8*4] where row
    v = (ix*128 + iy)*128 + iz holds the 2x2x2 neighborhood of cell
    (ix,iy,iz) (clamped at the upper edges).  One 64B row per query
    point -> ONE indirect-DMA descriptor per point instead of 8.
  - Shard the (padded) 2M points across 8 cores, 128 partitions x 2048
    points each.
  - Device: per tile of 128x256 points: compute cell index + fractional
    weights with ACT/DVE ops, gather rows with gpsimd indirect DMA,
    blend the 8 corners in fp16 on DVE, emit fp32.
"""
import numpy as np

import concourse.bacc as bacc
import concourse.bass as bass
import concourse.mybir as mybir
from concourse.bass_utils import run_bass_kernel_spmd
from concourse.tile import TileContext

NBINS = 128
OUTC = 4
CORES = 8
P = 128
TPP = 2048                    # points per partition per core
T = 256                       # tile: points per partition
NT = TPP // T
PTS_PER_CORE = P * TPP        # 262144
NPAD = CORES * PTS_PER_CORE   # 2097152
V = NBINS ** 3

F32 = mybir.dt.float32
F16 = mybir.dt.float16
I32 = mybir.dt.int32
AF = mybir.ActivationFunctionType
OP = mybir.AluOpType

# Exposed for the test harness: the BassKernelResults of the last run.
LAST_RESULT = None


def _build():
    nc = bacc.Bacc(None, target_bir_lowering=False)
    xs = nc.dram_tensor("xs", [P, TPP], F32, kind="ExternalInput")
    ys = nc.dram_tensor("ys", [P, TPP], F32, kind="ExternalInput")
    zs = nc.dram_tensor("zs", [P, TPP], F32, kind="ExternalInput")
    tab = nc.dram_tensor("tab", [V, 32], F16, kind="ExternalInput")
    out = nc.dram_tensor("out", [P, TPP, OUTC], F32, kind="ExternalOutput")

    with TileContext(nc) as tc:
        with tc.tile_pool(name="coords", bufs=1) as cpool, \
             tc.tile_pool(name="work", bufs=2) as pool, \
             tc.tile_pool(name="gbuf", bufs=2) as gpool:
            ct = []
            for name, src in (("xt", xs), ("yt", ys), ("zt", zs)):
                t = cpool.tile([P, TPP], F32, name=name)
                nc.sync.dma_start(out=t[:], in_=src[:])
                ct.append(t)

            for it in range(NT):
                sl = bass.ts(it, T)
                fa = []   # fractional parts (f32)
                ia = []   # integer parts as f32
                for ax in range(3):
                    # HW fp32->int32 cast is round-to-nearest-even, so
                    # floor(p) == rne(p - 0.5) (integer ties land one cell
                    # lower with f == 1.0 -- the same lerp, still in
                    # bounds thanks to the corner-expanded table).
                    ph_a = pool.tile([P, T], F32, name=f"ph{ax}")
                    nc.scalar.activation(out=ph_a[:], in_=ct[ax][:, sl],
                                         func=AF.Copy, scale=float(NBINS),
                                         bias=-0.5)
                    ii_a = pool.tile([P, T], I32, name=f"ii{ax}")
                    nc.vector.tensor_copy(out=ii_a[:], in_=ph_a[:])
                    i_a = pool.tile([P, T], F32, name=f"i{ax}")
                    nc.vector.tensor_copy(out=i_a[:], in_=ii_a[:])
                    f_a = pool.tile([P, T], F32, name=f"f{ax}")
                    nc.vector.scalar_tensor_tensor(out=f_a[:], in0=ph_a[:],
                                                   scalar=0.5, in1=i_a[:],
                                                   op0=OP.add, op1=OP.subtract)
                    fa.append(f_a)
                    ia.append(i_a)

                # idx = (ix*128 + iy)*128 + iz   (exact in f32), then int32
                t1 = pool.tile([P, T], F32, name="t1")
                nc.vector.scalar_tensor_tensor(out=t1[:], in0=ia[0][:],
                                               scalar=float(NBINS), in1=ia[1][:],
                                               op0=OP.mult, op1=OP.add)
                t2 = pool.tile([P, T], F32, name="t2")
                nc.vector.scalar_tensor_tensor(out=t2[:], in0=t1[:],
                                               scalar=float(NBINS), in1=ia[2][:],
                                               op0=OP.mult, op1=OP.add)
                idx = pool.tile([P, T], I32, name="idx")
                nc.vector.tensor_copy(out=idx[:], in_=t2[:])

                # gather the 2x2x2 neighborhoods: one 64B row per point.
                # The vector-indirect (multi-index) DMA form miscompiles in
                # this toolchain; only one index per partition per
                # instruction works, so issue T column gathers.
                G = gpool.tile([P, T, 32], F16, name="G")
                for j in range(T):
                    nc.gpsimd.indirect_dma_start(
                        out=G[:, j, :], out_offset=None,
                        in_=tab[:],
                        in_offset=bass.IndirectOffsetOnAxis(
                            ap=idx[:, j:j + 1], axis=0),
                    )

                # complements (1 - f) on ACT
                ca = []
                for ax in range(3):
                    c_a = pool.tile([P, T], F32, name=f"c{ax}")
                    nc.scalar.activation(out=c_a[:], in_=fa[ax][:],
                                         func=AF.Copy, scale=-1.0, bias=1.0)
                    ca.append(c_a)

                # corner weights w8[k], k = dx*4 + dy*2 + dz  (f32 -> f16)
                wxy = []
                for dx in range(2):
                    for dy in range(2):
                        w = pool.tile([P, T], F32, name=f"wxy{dx}{dy}")
                        nc.vector.tensor_tensor(
                            out=w[:],
                            in0=(fa[0] if dx else ca[0])[:],
                            in1=(fa[1] if dy else ca[1])[:],
                            op=OP.mult)
                        wxy.append(w)
                w16 = []
                for k in range(8):
                    dz = k & 1
                    w = pool.tile([P, T], F32, name=f"w8_{k}")
                    nc.vector.tensor_tensor(
                        out=w[:],
                        in0=wxy[k >> 1][:],
                        in1=(fa[2] if dz else ca[2])[:],
                        op=OP.mult)
                    w16_k = pool.tile([P, T], F16, name=f"w16_{k}")
                    nc.scalar.activation(out=w16_k[:], in_=w[:], func=AF.Copy)
                    w16.append(w16_k)

                # blend: acc[p,t,c] = sum_k w16[k] * G[:, :, 4k:4k+4]
                acc = pool.tile([P, T, OUTC], F16, name="acc")
                prod = pool.tile([P, T, OUTC], F16, name="prod")
                for k in range(8):
                    tgt = acc if k == 0 else prod
                    nc.vector.tensor_tensor(
                        out=tgt[:],
                        in0=G[:, :, 4 * k:4 * k + 4],
                        in1=w16[k][:].unsqueeze(-1).to_broadcast([P, T, OUTC]),
                        op=OP.mult)
                    if k > 0:
                        nc.vector.tensor_tensor(out=acc[:], in0=acc[:],
                                                in1=prod[:], op=OP.add)
                acc32 = pool.tile([P, T, OUTC], F32, name="acc32")
                nc.scalar.activation(out=acc32[:], in_=acc[:], func=AF.Copy)
                nc.sync.dma_start(out=out[:, sl, :], in_=acc32[:])
    nc.compile()
    return nc


def _build_table(grid: np.ndarray) -> np.ndarray:
    g = np.asarray(grid, dtype=np.float32)
    gp = np.pad(g, ((0, 1), (0, 1), (0, 1), (0, 0)), mode="edge")
    w = np.lib.stride_tricks.sliding_window_view(gp, (2, 2, 2), axis=(0, 1, 2))
    # w: [128,128,128, 4, 2,2,2] with trailing (dx,dy,dz); want (dx,dy,dz,c)
    e = w.transpose(0, 1, 2, 4, 5, 6, 3).reshape(V, 32)
    return np.ascontiguousarray(e, dtype=np.float16)


def kernel(x: np.ndarray, grid: np.ndarray) -> np.ndarray:
    global LAST_RESULT
    x = np.asarray(x, dtype=np.float32)
    n = x.shape[0]
    tab = _build_table(grid)

    xp = np.zeros((NPAD, 3), dtype=np.float32)
    xp[:n] = x
    xp = xp.reshape(CORES, P, TPP, 3)

    in_maps = []
    for c in range(CORES):
        in_maps.append({
            "xs": np.ascontiguousarray(xp[c, :, :, 0]),
            "ys": np.ascontiguousarray(xp[c, :, :, 1]),
            "zs": np.ascontiguousarray(xp[c, :, :, 2]),
            "tab": tab,
        })

    nc = _build()
    res = run_bass_kernel_spmd(nc, in_maps, core_ids=list(range(CORES)))
    LAST_RESULT = res
    out = np.concatenate(
        [np.asarray(r["out"], dtype=np.float32).reshape(PTS_PER_CORE, OUTC)
         for r in res.results], axis=0)
    return out[:n]



# revision 9
# speedup vs baseline: 1.0905x; 1.0905x over previous
"""Trilinear 3D grid-encoding lookup on 8 TRN2 NeuronCores.

Strategy (v3, sorted-window dma_gather):
  - Host: corner-expanded table E[v, 128] f16 (first 32 cols = the 2x2x2
    neighborhood of cell v, rest pad to a 256B row as required by the
    dma_gather stride).  Points are SORTED by cell index and binned into
    64 windows of 32768 table rows; core c owns windows [8c, 8c+8), one
    32768-point slot per window (padded with copies of a real point).
    Window-local indices fit int16 -> one InstDMAGatherAnt per 4096
    points instead of one indirect DMA per point.
  - Device: per half-slot tile of 128x128 points: compute cell index +
    fractional weights (ACT/DVE), subtract the core/slot window base,
    cast to int16, shuffle into the Q7 wrapped index layout via a DRAM
    bounce, gather 256B rows with dma_gather(single_packet=False), blend
    the 8 corners in f16, store f16 (host upcasts bit-identically).
  - Host: inverse-permute device outputs back to input order.
"""
import numpy as np

import concourse.bacc as bacc
import concourse.bass as bass
import concourse.mybir as mybir
from concourse.bass_utils import run_bass_kernel_spmd
from concourse.tile import TileContext

NBINS = 128
OUTC = 4
CORES = 8
P = 128
WSLOTS = 8                     # window slots per core
SLOT = 32768                   # points per slot (= int16 window rows)
HALF = SLOT // 2               # pipeline unit: 16384 points
TC = 128                       # columns per half-slot tile
TPP = WSLOTS * SLOT // P       # 2048 columns per partition
PTS_PER_CORE = WSLOTS * SLOT   # 262144
G = 4096                       # indices per dma_gather call
CALLS = HALF // G              # 4 calls per half-slot
V = NBINS ** 3
ROWF = 128                     # padded row: 128 f16 = 256B

F32 = mybir.dt.float32
F16 = mybir.dt.float16
I32 = mybir.dt.int32
I16 = mybir.dt.int16
AF = mybir.ActivationFunctionType
OP = mybir.AluOpType

LAST_RESULT = None


def _build():
    nc = bacc.Bacc(None, target_bir_lowering=False)
    xs = nc.dram_tensor("xs", [P, TPP], F32, kind="ExternalInput")
    ys = nc.dram_tensor("ys", [P, TPP], F32, kind="ExternalInput")
    zs = nc.dram_tensor("zs", [P, TPP], F32, kind="ExternalInput")
    tab = nc.dram_tensor("tab", [WSLOTS * SLOT, ROWF], F16, kind="ExternalInput")
    cbase = nc.dram_tensor("cbase", [P, 1], F32, kind="ExternalInput")
    # DRAM bounce for the idx wrap shuffle, one region per half-slot
    nhalf = WSLOTS * 2
    ibuf = nc.dram_tensor("ibuf", [nhalf, P * TC], I16, kind="Internal")
    out = nc.dram_tensor("out", [P, TPP, OUTC], F16, kind="ExternalOutput")

    with TileContext(nc) as tc:
        with tc.tile_pool(name="coords", bufs=1) as cpool, \
             tc.tile_pool(name="work", bufs=2) as pool, \
             tc.tile_pool(name="gbuf", bufs=2) as gpool:
            ct = []
            for name, src in (("xt", xs), ("yt", ys), ("zt", zs)):
                t = cpool.tile([P, TPP], F32, name=name)
                nc.sync.dma_start(out=t[:], in_=src[:])
                ct.append(t)
            cb = cpool.tile([P, 1], F32, name="cb")
            nc.sync.dma_start(out=cb[:], in_=cbase[:])

            for hh in range(nhalf):
                s = hh // 2                      # slot (window)
                col0 = hh * TC                   # first column of this half
                sl = bass.ts(hh, TC)
                fa = []   # fractional parts (f32)
                ia = []   # integer parts as f32
                for ax in range(3):
                    # floor(p) == rne(p - 0.5) on HW (f32->i32 cast is rne)
                    ph_a = pool.tile([P, TC], F32, name=f"ph{ax}")
                    nc.scalar.activation(out=ph_a[:], in_=ct[ax][:, sl],
                                         func=AF.Copy, scale=float(NBINS),
                                         bias=-0.5)
                    ii_a = pool.tile([P, TC], I32, name=f"ii{ax}")
                    nc.vector.tensor_copy(out=ii_a[:], in_=ph_a[:])
                    i_a = pool.tile([P, TC], F32, name=f"i{ax}")
                    nc.vector.tensor_copy(out=i_a[:], in_=ii_a[:])
                    f_a = pool.tile([P, TC], F32, name=f"f{ax}")
                    nc.vector.scalar_tensor_tensor(out=f_a[:], in0=ph_a[:],
                                                   scalar=0.5, in1=i_a[:],
                                                   op0=OP.add, op1=OP.subtract)
                    fa.append(f_a)
                    ia.append(i_a)

                # absolute cell idx = (ix*128 + iy)*128 + iz (exact in f32)
                t1 = pool.tile([P, TC], F32, name="t1")
                nc.vector.scalar_tensor_tensor(out=t1[:], in0=ia[0][:],
                                               scalar=float(NBINS), in1=ia[1][:],
                                               op0=OP.mult, op1=OP.add)
                t2 = pool.tile([P, TC], F32, name="t2")
                nc.vector.scalar_tensor_tensor(out=t2[:], in0=t1[:],
                                               scalar=float(NBINS), in1=ia[2][:],
                                               op0=OP.mult, op1=OP.add)
                # window-local: idx - s*32768 - cbase  (cbase = c*262144)
                tl = pool.tile([P, TC], F32, name="tl")
                nc.vector.scalar_tensor_tensor(
                    out=tl[:], in0=t2[:], scalar=float(s * SLOT),
                    in1=cb[:].to_broadcast([P, TC]),
                    op0=OP.subtract, op1=OP.subtract)
                tlr = pool.tile([P, TC], F32, name="tlr")
                nc.scalar.activation(out=tlr[:], in_=tl[:], func=AF.Relu)
                idx16 = pool.tile([P, TC], I16, name="idx16")
                nc.vector.tensor_copy(out=idx16[:], in_=tlr[:])

                # wrap shuffle via DRAM bounce.  Q7 reads index j of a call
                # from wrapped[j%16 (+16g), j//16]; global k = 16F + q.
                # w1 writes buf[q*1024 + F] = idx(point k=16F+q): for point
                # (p, i): k = i*128+p -> addr = (p%16)*1024 + i*8 + p//16.
                dst_ap = bass.AP(ibuf, hh * (P * TC),
                                 [[1, 8], [HALF // 16, 16], [8, TC]])
                nc.sync.dma_start(out=dst_ap, in_=idx16[:])
                # w2: read back replicated 8x: wrapped[16g+q, F] = buf[q*1024+F]
                wrapped = pool.tile([P, HALF // 16], I16, name="wrapped")
                src_ap = bass.AP(ibuf, hh * (P * TC),
                                 [[0, 8], [HALF // 16, 16], [1, HALF // 16]])
                nc.sync.dma_start(out=wrapped[:], in_=src_ap)

                # gather: CALLS x G rows of 256B into G_h [P, TC, ROWF]
                Gh = gpool.tile([P, TC, ROWF], F16, name="Gh")
                for t in range(CALLS):
                    nc.gpsimd.dma_gather(
                        out_ap=Gh[:, t * (G // P):(t + 1) * (G // P), :],
                        in_ap=tab[s * SLOT:(s + 1) * SLOT, :],
                        idxs_ap=wrapped[:, t * (G // 16):(t + 1) * (G // 16)],
                        num_idxs=G, num_idxs_reg=G, elem_size=ROWF,
                        single_packet=False)

                # weights
                ca = []
                for ax in range(3):
                    c_a = pool.tile([P, TC], F32, name=f"c{ax}")
                    nc.scalar.activation(out=c_a[:], in_=fa[ax][:],
                                         func=AF.Copy, scale=-1.0, bias=1.0)
                    ca.append(c_a)
                wxy = []
                for dx in range(2):
                    for dy in range(2):
                        w = pool.tile([P, TC], F32, name=f"wxy{dx}{dy}")
                        nc.vector.tensor_tensor(
                            out=w[:],
                            in0=(fa[0] if dx else ca[0])[:],
                            in1=(fa[1] if dy else ca[1])[:],
                            op=OP.mult)
                        wxy.append(w)
                w16 = []
                for k in range(8):
                    dz = k & 1
                    w16_k = pool.tile([P, TC], F16, name=f"w16_{k}")
                    nc.vector.tensor_tensor(
                        out=w16_k[:],
                        in0=wxy[k >> 1][:],
                        in1=(fa[2] if dz else ca[2])[:],
                        op=OP.mult)
                    w16.append(w16_k)

                # blend: acc[p,t,c] = sum_k w16[k] * Gh[:, :, 4k:4k+4]
                acc = pool.tile([P, TC, OUTC], F16, name="acc")
                prod = pool.tile([P, TC, OUTC], F16, name="prod")
                for k in range(8):
                    tgt = acc if k == 0 else prod
                    nc.vector.tensor_tensor(
                        out=tgt[:],
                        in0=Gh[:, :, 4 * k:4 * k + 4],
                        in1=w16[k][:].unsqueeze(-1).to_broadcast([P, TC, OUTC]),
                        op=OP.mult)
                    if k > 0:
                        nc.vector.tensor_tensor(out=acc[:], in0=acc[:],
                                                in1=prod[:], op=OP.add)
                nc.sync.dma_start(out=out[:, sl, :], in_=acc[:])
    nc.compile()
    return nc


def _build_table(grid: np.ndarray) -> np.ndarray:
    g = np.asarray(grid, dtype=np.float32)
    gp = np.pad(g, ((0, 1), (0, 1), (0, 1), (0, 0)), mode="edge")
    w = np.lib.stride_tricks.sliding_window_view(gp, (2, 2, 2), axis=(0, 1, 2))
    e = w.transpose(0, 1, 2, 4, 5, 6, 3).reshape(V, 32)
    tab = np.zeros((V, ROWF), dtype=np.float16)
    tab[:, :32] = e.astype(np.float16)
    return tab


def kernel(x: np.ndarray, grid: np.ndarray) -> np.ndarray:
    global LAST_RESULT
    x = np.asarray(x, dtype=np.float32)
    n = x.shape[0]
    tab = _build_table(grid)

    # replicate the device's floor-via-rne to bin points into windows
    ph = x * np.float32(NBINS) - np.float32(0.5)
    ii = np.rint(ph).astype(np.int32)          # rne like HW f32->i32
    cell = (ii[:, 0] * NBINS + ii[:, 1]) * NBINS + ii[:, 2]
    win = cell >> 15                            # 64 windows of 32768 rows

    order = np.argsort(win, kind="stable")
    counts = np.bincount(win, minlength=64)
    offs = np.zeros(65, dtype=np.int64)
    np.cumsum(counts, out=offs[1:])

    assign = np.empty((64, SLOT), dtype=np.int64)
    for w in range(64):
        pts = order[offs[w]:offs[w + 1]]
        if len(pts) > SLOT:
            raise RuntimeError(f"window {w} overflow: {len(pts)}")
        pad = pts[0] if len(pts) else order[0]
        assign[w, :len(pts)] = pts
        assign[w, len(pts):] = pad

    # device coord layout: core c, slot s, point j -> (part j%128, col s*256+j//128)
    in_maps = []
    for c in range(CORES):
        sel = assign[8 * c:8 * c + 8].reshape(-1)        # [8*SLOT]
        xc = x[sel].reshape(WSLOTS, SLOT // P, P, 3)      # [s, i, p, 3]
        xc = xc.transpose(2, 0, 1, 3).reshape(P, TPP, 3)  # [p, s*256+i, 3]
        in_maps.append({
            "xs": np.ascontiguousarray(xc[:, :, 0]),
            "ys": np.ascontiguousarray(xc[:, :, 1]),
            "zs": np.ascontiguousarray(xc[:, :, 2]),
            "tab": tab[8 * c * SLOT:(8 * c + 8) * SLOT],
            "cbase": np.full((P, 1), 8 * c * SLOT, dtype=np.float32),
        })

    nc = _build()
    res = run_bass_kernel_spmd(nc, in_maps, core_ids=list(range(CORES)))
    LAST_RESULT = res

    outp = np.empty((n, OUTC), dtype=np.float32)
    for c in range(CORES):
        dev = np.asarray(res.results[c]["out"], dtype=np.float32)  # [P, TPP, 4]
        vals = dev.reshape(P, WSLOTS, SLOT // P, OUTC)
        vals = vals.transpose(1, 2, 0, 3).reshape(WSLOTS * SLOT, OUTC)
        sel = assign[8 * c:8 * c + 8].reshape(-1)
        outp[sel] = vals
    return outp


# revision 12
# speedup vs baseline: 1.3746x; 1.2605x over previous
"""Trilinear 3D grid-encoding lookup on 8 TRN2 NeuronCores.

Strategy (v3, sorted-window dma_gather):
  - Host: corner-expanded table E[v, 128] f16 (first 32 cols = the 2x2x2
    neighborhood of cell v, rest pad to a 256B row as required by the
    dma_gather stride).  Points are SORTED by cell index and binned into
    64 windows of 32768 table rows; core c owns windows [8c, 8c+8), one
    32768-point slot per window (padded with copies of a real point).
    Window-local indices fit int16 -> one InstDMAGatherAnt per 4096
    points instead of one indirect DMA per point.
  - Device: per half-slot tile of 128x128 points: compute cell index +
    fractional weights (ACT/DVE), subtract the core/slot window base,
    cast to int16, shuffle into the Q7 wrapped index layout via a DRAM
    bounce, gather 256B rows with dma_gather(single_packet=False), blend
    the 8 corners in f16, store f16 (host upcasts bit-identically).
  - Host: inverse-permute device outputs back to input order.
"""
import numpy as np

import concourse.bacc as bacc
import concourse.bass as bass
import concourse.mybir as mybir
from concourse.bass_utils import run_bass_kernel_spmd
from concourse.tile import TileContext

NBINS = 128
OUTC = 4
CORES = 8
P = 128
WSLOTS = 8                     # window slots per core
SLOT = 32768                   # points per slot (= int16 window rows)
HALF = SLOT // 2               # pipeline unit: 16384 points
TC = 128                       # columns per half-slot tile
TPP = WSLOTS * SLOT // P       # 2048 columns per partition
PTS_PER_CORE = WSLOTS * SLOT   # 262144
G = 4096                       # indices per dma_gather call
CALLS = HALF // G              # 4 calls per half-slot
V = NBINS ** 3
ROWF = 128                     # padded row: 128 f16 = 256B

F32 = mybir.dt.float32
F16 = mybir.dt.float16
I32 = mybir.dt.int32
I16 = mybir.dt.int16
AF = mybir.ActivationFunctionType
OP = mybir.AluOpType

LAST_RESULT = None


def _build():
    nc = bacc.Bacc(None, target_bir_lowering=False)
    xs = nc.dram_tensor("xs", [P, TPP], F32, kind="ExternalInput")
    ys = nc.dram_tensor("ys", [P, TPP], F32, kind="ExternalInput")
    zs = nc.dram_tensor("zs", [P, TPP], F32, kind="ExternalInput")
    tab = nc.dram_tensor("tab", [WSLOTS * SLOT, ROWF], F16, kind="ExternalInput")
    cbase = nc.dram_tensor("cbase", [P, 1], F32, kind="ExternalInput")
    # DRAM bounce for the idx wrap shuffle, one region per half-slot
    nhalf = WSLOTS * 2
    ibuf = nc.dram_tensor("ibuf", [nhalf, P * TC], I16, kind="Internal")
    out = nc.dram_tensor("out", [P, TPP, OUTC], F16, kind="ExternalOutput")

    with TileContext(nc) as tc:
        with tc.tile_pool(name="coords", bufs=1) as cpool, \
             tc.tile_pool(name="work", bufs=2) as pool, \
             tc.tile_pool(name="gbuf", bufs=2) as gpool:
            ct = []
            for name, src in (("xt", xs), ("yt", ys), ("zt", zs)):
                t = cpool.tile([P, TPP], F32, name=name)
                nc.sync.dma_start(out=t[:], in_=src[:])
                ct.append(t)
            cb = cpool.tile([P, 1], F32, name="cb")
            nc.sync.dma_start(out=cb[:], in_=cbase[:])

            for hh in range(nhalf):
                s = hh // 2                      # slot (window)
                col0 = hh * TC                   # first column of this half
                sl = bass.ts(hh, TC)
                fa = []   # fractional parts (f32)
                ia = []   # integer parts as f32
                for ax in range(3):
                    # floor(p) == rne(p - 0.5) on HW (f32->i32 cast is rne)
                    ph_a = pool.tile([P, TC], F32, name=f"ph{ax}")
                    nc.scalar.activation(out=ph_a[:], in_=ct[ax][:, sl],
                                         func=AF.Copy, scale=float(NBINS),
                                         bias=-0.5)
                    ii_a = pool.tile([P, TC], I32, name=f"ii{ax}")
                    nc.vector.tensor_copy(out=ii_a[:], in_=ph_a[:])
                    i_a = pool.tile([P, TC], F32, name=f"i{ax}")
                    nc.vector.tensor_copy(out=i_a[:], in_=ii_a[:])
                    f_a = pool.tile([P, TC], F32, name=f"f{ax}")
                    nc.vector.scalar_tensor_tensor(out=f_a[:], in0=ph_a[:],
                                                   scalar=0.5, in1=i_a[:],
                                                   op0=OP.add, op1=OP.subtract)
                    fa.append(f_a)
                    ia.append(i_a)

                # absolute cell idx = (ix*128 + iy)*128 + iz (exact in f32)
                t1 = pool.tile([P, TC], F32, name="t1")
                nc.vector.scalar_tensor_tensor(out=t1[:], in0=ia[0][:],
                                               scalar=float(NBINS), in1=ia[1][:],
                                               op0=OP.mult, op1=OP.add)
                t2 = pool.tile([P, TC], F32, name="t2")
                nc.vector.scalar_tensor_tensor(out=t2[:], in0=t1[:],
                                               scalar=float(NBINS), in1=ia[2][:],
                                               op0=OP.mult, op1=OP.add)
                # window-local: idx - s*32768 - cbase  (cbase = c*262144)
                tl = pool.tile([P, TC], F32, name="tl")
                nc.vector.scalar_tensor_tensor(
                    out=tl[:], in0=t2[:], scalar=float(s * SLOT),
                    in1=cb[:].to_broadcast([P, TC]),
                    op0=OP.subtract, op1=OP.subtract)
                tlr = pool.tile([P, TC], F32, name="tlr")
                nc.scalar.activation(out=tlr[:], in_=tl[:], func=AF.Relu)
                idx16 = pool.tile([P, TC], I16, name="idx16")
                nc.vector.tensor_copy(out=idx16[:], in_=tlr[:])

                # wrap shuffle: the Q7 reads index j of a call from
                # wrapped[j%16 (+16g), j//16]; j = i*128 + p for point (p,i)
                # -> slot (q=p%16, F=i*8+p//16).  Route: DMA to DRAM in
                # (p_hi, q, i) layout (256B runs), DMA back replicated 8x as
                # [128, 8, TC] (2KB runs), then DVE-interleave in-partition.
                dst_ap = bass.AP(ibuf, hh * (P * TC),
                                 [[TC, 8], [8 * TC, 16], [1, TC]])
                nc.sync.dma_start(out=dst_ap, in_=idx16[:])
                tmpw = pool.tile([P, 8, TC], I16, name="tmpw")
                src_ap = bass.AP(ibuf, hh * (P * TC),
                                 [[0, 8], [8 * TC, 16], [1, 8 * TC]])
                nc.sync.dma_start(out=tmpw[:], in_=src_ap)
                wrapped = pool.tile([P, TC, 8], I16, name="wrapped")
                nc.vector.tensor_copy(out=wrapped[:],
                                      in_=tmpw[:].transpose([0, 2, 1]))

                # gather: CALLS x G rows of 256B into G_h [P, TC, ROWF]
                Gh = gpool.tile([P, TC, ROWF], F16, name="Gh")
                for t in range(CALLS):
                    nc.gpsimd.dma_gather(
                        out_ap=Gh[:, t * (G // P):(t + 1) * (G // P), :],
                        in_ap=tab[s * SLOT:(s + 1) * SLOT, :],
                        idxs_ap=wrapped[:, t * (G // P):(t + 1) * (G // P), :],
                        num_idxs=G, num_idxs_reg=G, elem_size=ROWF,
                        single_packet=False)

                # weights
                ca = []
                for ax in range(3):
                    c_a = pool.tile([P, TC], F32, name=f"c{ax}")
                    nc.scalar.activation(out=c_a[:], in_=fa[ax][:],
                                         func=AF.Copy, scale=-1.0, bias=1.0)
                    ca.append(c_a)
                wxy = []
                for dx in range(2):
                    for dy in range(2):
                        w = pool.tile([P, TC], F32, name=f"wxy{dx}{dy}")
                        nc.vector.tensor_tensor(
                            out=w[:],
                            in0=(fa[0] if dx else ca[0])[:],
                            in1=(fa[1] if dy else ca[1])[:],
                            op=OP.mult)
                        wxy.append(w)
                w16 = []
                for k in range(8):
                    dz = k & 1
                    w16_k = pool.tile([P, TC], F16, name=f"w16_{k}")
                    nc.vector.tensor_tensor(
                        out=w16_k[:],
                        in0=wxy[k >> 1][:],
                        in1=(fa[2] if dz else ca[2])[:],
                        op=OP.mult)
                    w16.append(w16_k)

                # blend: acc[p,t,c] = sum_k w16[k] * Gh[:, :, 4k:4k+4]
                acc = pool.tile([P, TC, OUTC], F16, name="acc")
                prod = pool.tile([P, TC, OUTC], F16, name="prod")
                for k in range(8):
                    tgt = acc if k == 0 else prod
                    nc.vector.tensor_tensor(
                        out=tgt[:],
                        in0=Gh[:, :, 4 * k:4 * k + 4],
                        in1=w16[k][:].unsqueeze(-1).to_broadcast([P, TC, OUTC]),
                        op=OP.mult)
                    if k > 0:
                        nc.vector.tensor_tensor(out=acc[:], in0=acc[:],
                                                in1=prod[:], op=OP.add)
                nc.sync.dma_start(out=out[:, sl, :], in_=acc[:])
    nc.compile()
    return nc


def _build_table(grid: np.ndarray) -> np.ndarray:
    g = np.asarray(grid, dtype=np.float32)
    gp = np.pad(g, ((0, 1), (0, 1), (0, 1), (0, 0)), mode="edge")
    w = np.lib.stride_tricks.sliding_window_view(gp, (2, 2, 2), axis=(0, 1, 2))
    e = w.transpose(0, 1, 2, 4, 5, 6, 3).reshape(V, 32)
    tab = np.zeros((V, ROWF), dtype=np.float16)
    tab[:, :32] = e.astype(np.float16)
    return tab


def kernel(x: np.ndarray, grid: np.ndarray) -> np.ndarray:
    global LAST_RESULT
    x = np.asarray(x, dtype=np.float32)
    n = x.shape[0]
    tab = _build_table(grid)

    # replicate the device's floor-via-rne to bin points into windows
    ph = x * np.float32(NBINS) - np.float32(0.5)
    ii = np.rint(ph).astype(np.int32)          # rne like HW f32->i32
    cell = (ii[:, 0] * NBINS + ii[:, 1]) * NBINS + ii[:, 2]
    win = cell >> 15                            # 64 windows of 32768 rows

    order = np.argsort(win, kind="stable")
    counts = np.bincount(win, minlength=64)
    offs = np.zeros(65, dtype=np.int64)
    np.cumsum(counts, out=offs[1:])

    assign = np.empty((64, SLOT), dtype=np.int64)
    for w in range(64):
        pts = order[offs[w]:offs[w + 1]]
        if len(pts) > SLOT:
            raise RuntimeError(f"window {w} overflow: {len(pts)}")
        pad = pts[0] if len(pts) else order[0]
        assign[w, :len(pts)] = pts
        assign[w, len(pts):] = pad

    # device coord layout: core c, slot s, point j -> (part j%128, col s*256+j//128)
    in_maps = []
    for c in range(CORES):
        sel = assign[8 * c:8 * c + 8].reshape(-1)        # [8*SLOT]
        xc = x[sel].reshape(WSLOTS, SLOT // P, P, 3)      # [s, i, p, 3]
        xc = xc.transpose(2, 0, 1, 3).reshape(P, TPP, 3)  # [p, s*256+i, 3]
        in_maps.append({
            "xs": np.ascontiguousarray(xc[:, :, 0]),
            "ys": np.ascontiguousarray(xc[:, :, 1]),
            "zs": np.ascontiguousarray(xc[:, :, 2]),
            "tab": tab[8 * c * SLOT:(8 * c + 8) * SLOT],
            "cbase": np.full((P, 1), 8 * c * SLOT, dtype=np.float32),
        })

    nc = _build()
    res = run_bass_kernel_spmd(nc, in_maps, core_ids=list(range(CORES)))
    LAST_RESULT = res

    outp = np.empty((n, OUTC), dtype=np.float32)
    for c in range(CORES):
        dev = np.asarray(res.results[c]["out"], dtype=np.float32)  # [P, TPP, 4]
        vals = dev.reshape(P, WSLOTS, SLOT // P, OUTC)
        vals = vals.transpose(1, 2, 0, 3).reshape(WSLOTS * SLOT, OUTC)
        sel = assign[8 * c:8 * c + 8].reshape(-1)
        outp[sel] = vals
    return outp
